# revision 61
# baseline (speedup 1.0000x reference)
import numpy as np

# nn_Attention: B=256, N=65, DIM=1024, HEADS=16, DH=64 across 8 cores (32 batches/core)
B, N, DIM, HEADS, DH = 256, 65, 1024, 16, 64
NCORES = 8
BPC = B // NCORES            # 32 batches per core
TOK = BPC * N                # 2080 tokens per core
CH = 416                     # token-chunk for the qkv projection
NCHUNK = TOK // CH           # 5
BN_EPS = 1e-5


def _head_of(g, i4):
    # exp-group g holds 4 heads: g0={0,2,4,6} g1={1,3,5,7} g2={8,10,..} g3={9,11,..}
    if g < 2:
        return g + 2 * i4
    return 8 + (g - 2) + 2 * i4


def _excol(h):
    # column of head h inside the [N, 4*260] ex/exn/rec tiles
    if h < 8:
        return (h % 2) * 260 + (h // 2) * 65
    return 520 + (h % 2) * 260 + ((h - 8) // 2) * 65


def _build(nc_mod, mybir, bass, has_bqkv, has_bout):
    f32 = mybir.dt.float32
    f32r = mybir.dt.float32r
    bf16 = mybir.dt.bfloat16
    Alu = mybir.AluOpType
    Act = mybir.ActivationFunctionType
    from concourse.tile import TileContext
    from contextlib import ExitStack

    nc = nc_mod
    xt = nc.declare_dram_parameter("xt", [DIM, TOK], f32r, isOutput=False)
    wqkvt = nc.declare_dram_parameter("wqkvt", [DIM, 3 * DIM], f32r, isOutput=False)
    woutt = nc.declare_dram_parameter("woutt", [DIM, DIM], f32r, isOutput=False)
    wconvt = nc.declare_dram_parameter("wconvt", [9, N, N], f32r, isOutput=False)
    stp = nc.declare_dram_parameter("st", [N, 2], f32, isOutput=False)
    idp = nc.declare_dram_parameter("ident", [128, 128], bf16, isOutput=False)
    idfp = nc.declare_dram_parameter("identf", [N, N], f32, isOutput=False)
    if has_bqkv:
        bqp = nc.declare_dram_parameter("bqkvc", [128, 24], f32, isOutput=False)
    if has_bout:
        bop = nc.declare_dram_parameter("boutc", [1, DIM], f32r, isOutput=False)
    out = nc.declare_dram_parameter("out", [TOK, DIM], f32, isOutput=True)

    R = lambda ap: ap

    with TileContext(nc) as tc:
        with ExitStack() as es:
            P = lambda *a, **k: es.enter_context(tc.tile_pool(*a, **k))
            cp = P(name="consts", bufs=1)
            qkvp = P(name="qkv", bufs=1)

            id_sb = cp.tile([128, 128], bf16, tag="id")
            nc.sync.dma_start(out=id_sb[:], in_=idp[:])
            idf_sb = cp.tile([N, N], f32, tag="idf")
            nc.sync.dma_start(out=idf_sb[:], in_=idfp[:])
            wconv_sb = cp.tile([N, 9 * N], f32r, tag="wconv")
            nc.sync.dma_start(
                out=wconv_sb[:].rearrange("c (t o) -> c t o", t=9),
                in_=wconvt[:].rearrange("t c o -> c t o"),
            )
            wconv = wconv_sb[:].rearrange("c (t o) -> c t o", t=9)
            st_sb = cp.tile([N, 2], f32, tag="st")
            nc.sync.dma_start(out=st_sb[:], in_=stp[:])
            ones65 = cp.tile([N, N], f32r, tag="ones65")
            nc.gpsimd.memset(ones65[:].bitcast(f32), 1.0)
            if has_bqkv:
                bq_sb = cp.tile([128, 24], f32, tag="bq")
                nc.sync.dma_start(out=bq_sb[:], in_=bqp[:])
            if has_bout:
                bo_sb = cp.tile([1, DIM], f32r, tag="bo")
                nc.sync.dma_start(out=bo_sb[:], in_=bop[:])
                ones1 = cp.tile([1, N], f32r, tag="ones1")
                nc.gpsimd.memset(ones1[:].bitcast(f32), 1.0)

            q_sb = qkvp.tile([128, 8 * TOK], bf16, tag="q")
            k_sb = qkvp.tile([128, 8 * TOK], bf16, tag="k")
            v_sb = qkvp.tile([128, 8 * TOK], bf16, tag="v")
            qv = q_sb[:].rearrange("p (a n) -> p a n", a=8)
            kv = k_sb[:].rearrange("p (a n) -> p a n", a=8)
            vv = v_sb[:].rearrange("p (a n) -> p a n", a=8)

            # ---- phase 1: qkv projection (feature-major, f32r), xt scoped ----
            with ExitStack() as es1:
                P1 = lambda *a, **k: es1.enter_context(tc.tile_pool(*a, **k))
                xtp = P1(name="xtp", bufs=1)
                wqp = P1(name="wqp", bufs=3)
                pP = P1(name="pP", bufs=5, space="PSUM")
                xt_sb = xtp.tile([128, 8 * TOK], f32r, tag="xt")
                xtv = xt_sb[:].rearrange("p (a n) -> p a n", a=8)
                xtd = xt[:].rearrange("(a p) n -> p a n", p=128)
                nc.sync.dma_start(
                    out=xtv[:, 0:2, 0:CH], in_=xtd[:, 0:2, 0:CH])
                wqd = wqkvt[:].rearrange("(ki p) f -> p ki f", p=128)

                for di, (dv, coff, boff) in enumerate(
                        ((vv, 2 * DIM, 16), (kv, DIM, 8), (qv, 0, 0))):
                    for a in range(8):
                        wqb = wqp.tile([128, 8 * 128], f32r, tag="wqb")
                        wqbv = wqb[:].rearrange("p (ki f) -> p ki f", ki=8)
                        nc.sync.dma_start(
                            out=wqbv,
                            in_=wqd[:, :, coff + a * 128:coff + (a + 1) * 128],
                        )
                        if di == 0 and a == 0:
                            # rest of x streams in behind the first weight block
                            nc.sync.dma_start(
                                out=xtv[:, 2:5, 0:CH], in_=xtd[:, 2:5, 0:CH])
                            nc.sync.dma_start(
                                out=xtv[:, 5:8, 0:CH], in_=xtd[:, 5:8, 0:CH])
                            for c in range(1, NCHUNK):
                                nc.sync.dma_start(
                                    out=xtv[:, :, c * CH:(c + 1) * CH],
                                    in_=xtd[:, :, c * CH:(c + 1) * CH],
                                )
                        for c in range(NCHUNK):
                            ps = pP.tile([128, CH], f32, tag="pp")
                            for ki in range(8):
                                nc.tensor.matmul(
                                    ps[:],
                                    R(wqbv[:, ki, :]),
                                    R(xtv[:, ki, c * CH:(c + 1) * CH]),
                                    start=(ki == 0), stop=(ki == 7),
                                )
                            o = dv[:, a, c * CH:(c + 1) * CH]
                            if has_bqkv:
                                nc.vector.tensor_scalar_add(
                                    o, ps[:], bq_sb[:, boff + a:boff + a + 1])
                            elif (a + c) % 2 == 0:
                                nc.vector.tensor_copy(o, ps[:])
                            else:
                                nc.scalar.copy(o, ps[:])

            # ---- phase 2: attention + conv + out projection per batch ----
            wop = P(name="wop", bufs=1)
            wo_sb = wop.tile([128, 8 * DIM], f32r, tag="wo")
            nc.sync.dma_start(
                out=wo_sb[:].rearrange("p (a n) -> p a n", a=8),
                in_=woutt[:].rearrange("(a p) n -> p a n", p=128),
            )
            wo = wo_sb[:].rearrange("p (a n) -> p a n", a=8)

            vphp = P(name="vphp", bufs=3)
            vpfp = P(name="vpfp", bufs=3)
            exp_ = P(name="exp", bufs=3)
            exnp = P(name="exnp", bufs=3)
            recp = P(name="recp", bufs=3)
            rtp = P(name="rtp", bufs=3)
            btp = P(name="btp", bufs=3)
            bnp = P(name="bnp", bufs=3)
            obp = P(name="obp", bufs=3)
            pMED = P(name="pmed", bufs=2, space="PSUM")
            pPT = P(name="ppt", bufs=2, space="PSUM")
            pAC = P(name="pac", bufs=2, space="PSUM")
            pBIG = P(name="pbig", bufs=2, space="PSUM")

            for b in range(BPC):
                toff = b * N
                # v for this batch, token-major padded image [c=65, 16 rows, 66]
                # bf16 copy feeds attn@v, f32 copy feeds the f32r conv
                vph = vphp.tile([N, HEADS * 66], bf16, tag="vph")
                vphh = vph[:].rearrange("c (h w) -> c h w", w=66)
                vpf = vpfp.tile([N, HEADS * 66], f32r, tag="vpf")
                vpfh = vpf[:].rearrange("c (h w) -> c h w", w=66)
                nc.gpsimd.memset(vphh[:, :, 0:1], 0.0)
                nc.gpsimd.memset(vphh[:, :, 65:66], 0.0)
                nc.gpsimd.memset(vpfh[:, :, 0:1].bitcast(f32), 0.0)
                nc.gpsimd.memset(vpfh[:, :, 65:66].bitcast(f32), 0.0)
                for a in range(8):
                    pt = pPT.tile([128, 128], bf16, tag="pt")
                    nc.tensor.transpose(pt[0:N, :], vv[:, a, toff:toff + N], id_sb[:])
                    src = pt[0:N, :].rearrange("c (h d) -> c h d", d=64)
                    if a % 2 == 0:
                        nc.vector.tensor_copy(vphh[:, 2 * a:2 * a + 2, 1:65], src)
                        nc.scalar.copy(vpfh[:, 2 * a:2 * a + 2, 1:65], src)
                    else:
                        nc.scalar.copy(vphh[:, 2 * a:2 * a + 2, 1:65], src)
                        nc.vector.tensor_copy(vpfh[:, 2 * a:2 * a + 2, 1:65], src)

                # attention scores, exp, rowsums (PE broadcast), recip, normalize
                ex = exp_.tile([N, 4 * 260], f32r, tag="ex")
                exn = exnp.tile([N, 4 * 260], bf16, tag="exn")
                rec = recp.tile([N, 4 * 260], f32, tag="rec")
                for g in range(4):
                    pd = pMED.tile([N, 260], f32, tag="med")
                    for i4 in range(4):
                        h = _head_of(g, i4)
                        a, ph = h // 2, (h % 2) * 64
                        nc.tensor.matmul(
                            pd[:, i4 * 65:(i4 + 1) * 65],
                            kv[ph:ph + 64, a, toff:toff + N],
                            qv[ph:ph + 64, a, toff:toff + N],
                            start=True, stop=True,
                        )
                    gc = slice(g * 260, (g + 1) * 260)
                    nc.scalar.activation(ex[:, gc], pd[:], Act.Exp)
                # conv 3x3 SAME (f32r, no ldweights) + BN affine
                bn = bnp.tile([N, DIM], f32, tag="bn")
                pcs = [pBIG.tile([128, 512], f32, tag="big", name="pcs")
                       for _ in range(2)]
                pcys = [p[0:N, :].rearrange("c (h d) -> c h d", d=64) for p in pcs]
                for t in (4, 0, 1, 2, 3, 5, 6, 7, 8):
                    dy, dx = t // 3 - 1, t % 3 - 1
                    for half in range(2):
                        y0, y1 = half * 8, half * 8 + 8
                        oy0, oy1 = max(y0, -dy), min(y1, 16 - dy)
                        nc.tensor.matmul(
                            pcys[half][:, oy0 - y0:oy1 - y0, :],
                            R(wconv[:, t, :]),
                            R(vpfh[:, oy0 + dy:oy1 + dy, 1 + dx:65 + dx]),
                            start=(t == 4), stop=(t == 8),
                        )

                for g in range(4):
                    gc = slice(g * 260, (g + 1) * 260)
                    sm = pMED.tile([N, 260], f32, tag="med")
                    nc.tensor.matmul(sm[:], R(ones65[:]), R(ex[:, gc]),
                                     start=True, stop=True)
                    nc.vector.reciprocal(rec[:, gc], sm[:])
                    if g < 2:
                        nc.gpsimd.tensor_tensor(exn[:, gc], ex[:, gc], rec[:, gc],
                                                Alu.mult)
                    else:
                        nc.vector.tensor_tensor(exn[:, gc], ex[:, gc], rec[:, gc],
                                                Alu.mult)

                for half in range(2):
                    nc.vector.tensor_scalar(
                        bn[:, half * 512:half * 512 + 512], pcs[half][0:N, :],
                        st_sb[:, 0:1], st_sb[:, 1:2], Alu.mult, Alu.add,
                    )
                # fused: rt = (attn@v)^T + bn^T  per head pair
                rt = rtp.tile([128, 8 * N], f32r, tag="rt")
                for c8 in range(8):
                    bt = pAC.tile([128, 128], f32, tag="ac")
                    nc.tensor.matmul(
                        R(bt[:, 0:N]), R(bn[:, c8 * 128:(c8 + 1) * 128]),
                        R(idf_sb[:]),
                        is_transpose=True, start=True, stop=True,
                    )
                    btf = btp.tile([128, N], f32, tag="btf")
                    nc.scalar.copy(btf[:], bt[:, 0:N])
                    ac = pAC.tile([128, 128], f32, tag="ac")
                    he, ho = 2 * c8, 2 * c8 + 1
                    ce, co = _excol(he), _excol(ho)
                    nc.tensor.matmul(
                        ac[0:64, 0:N], vphh[:, he, 1:65], exn[:, ce:ce + 65],
                        start=True, stop=True,
                    )
                    nc.tensor.matmul(
                        ac[64:128, 0:N], vphh[:, ho, 1:65], exn[:, co:co + 65],
                        start=True, stop=True,
                    )
                    nc.vector.tensor_tensor(
                        rt[:, c8 * N:(c8 + 1) * N], ac[:, 0:N], btf[:], Alu.add)

                # final projection [65, 1024], f32r (no ldweights)
                pos = [pBIG.tile([128, 512], f32, tag="big", name="pos")
                       for _ in range(2)]
                for ki in range(8):
                    for half in range(2):
                        nc.tensor.matmul(
                            pos[half][0:N, :],
                            R(rt[:, ki * N:(ki + 1) * N]),
                            R(wo[:, ki, half * 512:half * 512 + 512]),
                            start=(ki == 0), stop=(ki == 7 and not has_bout),
                        )
                if has_bout:
                    for half in range(2):
                        nc.tensor.matmul(
                            pos[half][0:N, :], R(ones1[:]),
                            R(bo_sb[:, half * 512:half * 512 + 512]),
                            start=False, stop=True,
                        )
                for half in range(2):
                    ob = obp.tile([N, 512], f32, tag="ob")
                    if half == 0:
                        nc.vector.tensor_copy(ob[:], pos[half][0:N, :])
                    else:
                        nc.scalar.copy(ob[:], pos[half][0:N, :])
                    nc.sync.dma_start(
                        out=out[toff:toff + N, half * 512:half * 512 + 512],
                        in_=ob[:],
                    )
    return nc


def kernel(x, w_qkv, b_qkv, w_out, b_out, conv_w, conv_b,
           bn_gamma, bn_beta, bn_mean, bn_var):
    import os
    os.environ["BASS_NEVER_TRACE"] = "1"   # no NTFF hook in this container
    import concourse.bass as bass
    import concourse.bacc as bacc
    import concourse.mybir as mybir
    from concourse.bass_utils import run_bass_kernel_spmd
    import ml_dtypes

    bf = ml_dtypes.bfloat16
    scale = float(DIM) ** -0.5
    x = np.asarray(x, np.float32)
    wq = np.asarray(w_qkv, np.float32).copy()
    wq[:DIM] *= scale                                  # fold attn scale into W_q
    wqkvt = np.ascontiguousarray(wq.T)                 # [1024, 3072] f32
    woutt = np.ascontiguousarray(np.asarray(w_out, np.float32).T)
    wconvt = np.ascontiguousarray(
        np.asarray(conv_w, np.float32).transpose(2, 3, 1, 0).reshape(9, N, N))
    s = np.asarray(bn_gamma, np.float32) / np.sqrt(
        np.asarray(bn_var, np.float32) + BN_EPS)
    t_aff = (np.asarray(conv_b, np.float32) - np.asarray(bn_mean, np.float32)) * s \
        + np.asarray(bn_beta, np.float32)
    st = np.ascontiguousarray(np.stack([s, t_aff], 1))

    b_qkv = np.asarray(b_qkv, np.float32)
    b_out = np.asarray(b_out, np.float32)
    has_bqkv = bool(np.any(b_qkv))
    has_bout = bool(np.any(b_out))

    nc = bacc.Bacc()
    _build(nc, mybir, bass, has_bqkv, has_bout)
    nc.finalize()

    xt_all = np.ascontiguousarray(x.reshape(B * N, DIM).T)

    base = {"wqkvt": wqkvt, "woutt": woutt, "wconvt": wconvt, "st": st,
            "ident": np.eye(128, dtype=bf),
            "identf": np.eye(N, dtype=np.float32)}
    if has_bqkv:
        bq = b_qkv.copy()
        bq[:DIM] *= scale
        base["bqkvc"] = np.ascontiguousarray(bq.reshape(24, 128).T)
    if has_bout:
        base["boutc"] = np.ascontiguousarray(b_out[None, :])

    in_maps = []
    for c in range(NCORES):
        m = dict(base)
        m["xt"] = np.ascontiguousarray(xt_all[:, c * TOK:(c + 1) * TOK])
        in_maps.append(m)
    res = run_bass_kernel_spmd(nc, in_maps, list(range(NCORES)))
    globals()["LAST_RESULT"] = res
    outs = [res.results[c]["out"] for c in range(NCORES)]
    return np.concatenate(outs, axis=0).reshape(B, N, DIM).astype(np.float32)


# revision 66
# speedup vs baseline: 1.0806x; 1.0806x over previous
import numpy as np

# nn_Attention: B=256, N=65, DIM=1024, HEADS=16, DH=64 across 8 cores (32 batches/core)
B, N, DIM, HEADS, DH = 256, 65, 1024, 16, 64
NCORES = 8
BPC = B // NCORES            # 32 batches per core
TOK = BPC * N                # 2080 tokens per core
CH = 416                     # token-chunk for the qkv projection
NCHUNK = TOK // CH           # 5
BN_EPS = 1e-5


def _head_of(g, i4):
    # exp-group g holds 4 heads: g0={0,2,4,6} g1={1,3,5,7} g2={8,10,..} g3={9,11,..}
    if g < 2:
        return g + 2 * i4
    return 8 + (g - 2) + 2 * i4


def _excol(h):
    # column of head h inside the [N, 4*260] ex/exn/rec tiles
    if h < 8:
        return (h % 2) * 260 + (h // 2) * 65
    return 520 + (h % 2) * 260 + ((h - 8) // 2) * 65


def _build(nc_mod, mybir, bass, has_bqkv, has_bout):
    f32 = mybir.dt.float32
    f32r = mybir.dt.float32r
    bf16 = mybir.dt.bfloat16
    Alu = mybir.AluOpType
    Act = mybir.ActivationFunctionType
    from concourse.tile import TileContext
    from contextlib import ExitStack

    nc = nc_mod
    xt = nc.declare_dram_parameter("xt", [DIM, TOK], f32r, isOutput=False)
    wqkvt = nc.declare_dram_parameter("wqkvt", [DIM, 3 * DIM], f32r, isOutput=False)
    woutt = nc.declare_dram_parameter("woutt", [DIM, DIM], bf16, isOutput=False)
    wconvt = nc.declare_dram_parameter("wconvt", [9, N, N], f32r, isOutput=False)
    stp = nc.declare_dram_parameter("st", [N, 2], f32, isOutput=False)
    idp = nc.declare_dram_parameter("ident", [128, 128], bf16, isOutput=False)
    idfp = nc.declare_dram_parameter("identf", [N, N], f32, isOutput=False)
    if has_bqkv:
        bqp = nc.declare_dram_parameter("bqkvc", [128, 24], f32, isOutput=False)
    if has_bout:
        bop = nc.declare_dram_parameter("boutc", [1, DIM], bf16, isOutput=False)
    out = nc.declare_dram_parameter("out", [TOK, DIM], f32, isOutput=True)

    R = lambda ap: ap

    with TileContext(nc) as tc:
        with ExitStack() as es:
            P = lambda *a, **k: es.enter_context(tc.tile_pool(*a, **k))
            cp = P(name="consts", bufs=1)
            qkvp = P(name="qkv", bufs=1)

            id_sb = cp.tile([128, 128], bf16, tag="id")
            nc.sync.dma_start(out=id_sb[:], in_=idp[:])
            idf_sb = cp.tile([N, N], f32, tag="idf")
            nc.sync.dma_start(out=idf_sb[:], in_=idfp[:])
            wconv_sb = cp.tile([N, 9 * N], f32r, tag="wconv")
            nc.sync.dma_start(
                out=wconv_sb[:].rearrange("c (t o) -> c t o", t=9),
                in_=wconvt[:].rearrange("t c o -> c t o"),
            )
            wconv = wconv_sb[:].rearrange("c (t o) -> c t o", t=9)
            st_sb = cp.tile([N, 2], f32, tag="st")
            nc.sync.dma_start(out=st_sb[:], in_=stp[:])
            ones65 = cp.tile([N, N], f32r, tag="ones65")
            nc.gpsimd.memset(ones65[:].bitcast(f32), 1.0)
            if has_bqkv:
                bq_sb = cp.tile([128, 24], f32, tag="bq")
                nc.sync.dma_start(out=bq_sb[:], in_=bqp[:])
            if has_bout:
                bo_sb = cp.tile([1, DIM], bf16, tag="bo")
                nc.sync.dma_start(out=bo_sb[:], in_=bop[:])
                ones1 = cp.tile([1, 128], bf16, tag="ones1")
                nc.gpsimd.memset(ones1[:], 1.0)

            q_sb = qkvp.tile([128, 8 * TOK], bf16, tag="q")
            k_sb = qkvp.tile([128, 8 * TOK], bf16, tag="k")
            v_sb = qkvp.tile([128, 8 * TOK], bf16, tag="v")
            qv = q_sb[:].rearrange("p (a n) -> p a n", a=8)
            kv = k_sb[:].rearrange("p (a n) -> p a n", a=8)
            vv = v_sb[:].rearrange("p (a n) -> p a n", a=8)

            # ---- phase 1: qkv projection (feature-major, f32r), xt scoped ----
            with ExitStack() as es1:
                P1 = lambda *a, **k: es1.enter_context(tc.tile_pool(*a, **k))
                xtp = P1(name="xtp", bufs=1)
                wqp = P1(name="wqp", bufs=3)
                pP = P1(name="pP", bufs=5, space="PSUM")
                xt_sb = xtp.tile([128, 8 * TOK], f32r, tag="xt")
                xtv = xt_sb[:].rearrange("p (a n) -> p a n", a=8)
                xtd = xt[:].rearrange("(a p) n -> p a n", p=128)
                nc.sync.dma_start(
                    out=xtv[:, 0:2, 0:CH], in_=xtd[:, 0:2, 0:CH])
                wqd = wqkvt[:].rearrange("(ki p) f -> p ki f", p=128)

                for di, (dv, coff, boff) in enumerate(
                        ((vv, 2 * DIM, 16), (kv, DIM, 8), (qv, 0, 0))):
                    for a in range(8):
                        wqb = wqp.tile([128, 8 * 128], f32r, tag="wqb")
                        wqbv = wqb[:].rearrange("p (ki f) -> p ki f", ki=8)
                        nc.sync.dma_start(
                            out=wqbv,
                            in_=wqd[:, :, coff + a * 128:coff + (a + 1) * 128],
                        )
                        if di == 0 and a == 0:
                            # rest of x streams in behind the first weight block
                            nc.sync.dma_start(
                                out=xtv[:, 2:5, 0:CH], in_=xtd[:, 2:5, 0:CH])
                            nc.sync.dma_start(
                                out=xtv[:, 5:8, 0:CH], in_=xtd[:, 5:8, 0:CH])
                            for c in range(1, NCHUNK):
                                nc.sync.dma_start(
                                    out=xtv[:, :, c * CH:(c + 1) * CH],
                                    in_=xtd[:, :, c * CH:(c + 1) * CH],
                                )
                        for c in range(NCHUNK):
                            ps = pP.tile([128, CH], f32, tag="pp")
                            for ki in range(8):
                                nc.tensor.matmul(
                                    ps[:],
                                    R(wqbv[:, ki, :]),
                                    R(xtv[:, ki, c * CH:(c + 1) * CH]),
                                    start=(ki == 0), stop=(ki == 7),
                                )
                            o = dv[:, a, c * CH:(c + 1) * CH]
                            if has_bqkv:
                                nc.vector.tensor_scalar_add(
                                    o, ps[:], bq_sb[:, boff + a:boff + a + 1])
                            elif (a + c) % 2 == 0:
                                nc.vector.tensor_copy(o, ps[:])
                            else:
                                nc.scalar.copy(o, ps[:])

            # ---- phase 2: attention + conv + out projection per batch ----
            wop = P(name="wop", bufs=1)
            wo_sb = wop.tile([128, 8 * DIM], bf16, tag="wo")
            nc.sync.dma_start(
                out=wo_sb[:].rearrange("p (a n) -> p a n", a=8),
                in_=woutt[:].rearrange("(a p) n -> p a n", p=128),
            )
            wo = wo_sb[:].rearrange("p (a n) -> p a n", a=8)

            vphp = P(name="vphp", bufs=3)
            vpfp = P(name="vpfp", bufs=2)
            exp_ = P(name="exp", bufs=2)
            exnp = P(name="exnp", bufs=3)
            recp = P(name="recp", bufs=3)
            rtp = P(name="rtp", bufs=1)
            btp = P(name="btp", bufs=3)
            bnp = P(name="bnp", bufs=2)
            obp = P(name="obp", bufs=3)
            rta = rtp.tile([128, 8 * TOK], bf16, tag="rta")
            rtv = rta[:].rearrange("p (a n) -> p a n", a=8)
            pMED = P(name="pmed", bufs=2, space="PSUM")
            pPT = P(name="ppt", bufs=2, space="PSUM")
            pAC = P(name="pac", bufs=2, space="PSUM")
            pBIG = P(name="pbig", bufs=2, space="PSUM")

            def emit_out_group(r0, M):
                for half in range(2):
                    po = pBIG.tile([128, 512], f32, tag="big", name="po")
                    for ki in range(8):
                        nc.tensor.matmul(
                            po[0:M, :],
                            rtv[:, ki, r0:r0 + M],
                            wo[:, ki, half * 512:half * 512 + 512],
                            start=(ki == 0), stop=(ki == 7 and not has_bout),
                        )
                    if has_bout:
                        nc.tensor.matmul(
                            po[0:M, :], ones1[:, 0:M],
                            bo_sb[:, half * 512:half * 512 + 512],
                            start=False, stop=True,
                        )
                    ob = obp.tile([128, 512], f32, tag="ob")
                    if half == 0:
                        nc.vector.tensor_copy(ob[0:M, :], po[0:M, :])
                    else:
                        nc.scalar.copy(ob[0:M, :], po[0:M, :])
                    nc.sync.dma_start(
                        out=out[r0:r0 + M, half * 512:half * 512 + 512],
                        in_=ob[0:M, :],
                    )

            gdone = 0
            for b in range(BPC):
                toff = b * N
                # v for this batch, token-major padded image [c=65, 16 rows, 66]
                # bf16 copy feeds attn@v, f32 copy feeds the f32r conv
                vph = vphp.tile([N, HEADS * 66], bf16, tag="vph")
                vphh = vph[:].rearrange("c (h w) -> c h w", w=66)
                vpf = vpfp.tile([N, HEADS * 66], f32r, tag="vpf")
                vpfh = vpf[:].rearrange("c (h w) -> c h w", w=66)
                nc.gpsimd.memset(vphh[:, :, 0:1], 0.0)
                nc.gpsimd.memset(vphh[:, :, 65:66], 0.0)
                nc.gpsimd.memset(vpfh[:, :, 0:1].bitcast(f32), 0.0)
                nc.gpsimd.memset(vpfh[:, :, 65:66].bitcast(f32), 0.0)
                for a in range(8):
                    pt = pPT.tile([128, 128], bf16, tag="pt")
                    nc.tensor.transpose(pt[0:N, :], vv[:, a, toff:toff + N], id_sb[:])
                    src = pt[0:N, :].rearrange("c (h d) -> c h d", d=64)
                    if a % 2 == 0:
                        nc.vector.tensor_copy(vphh[:, 2 * a:2 * a + 2, 1:65], src)
                        nc.scalar.copy(vpfh[:, 2 * a:2 * a + 2, 1:65], src)
                    else:
                        nc.scalar.copy(vphh[:, 2 * a:2 * a + 2, 1:65], src)
                        nc.vector.tensor_copy(vpfh[:, 2 * a:2 * a + 2, 1:65], src)

                # attention scores, exp, rowsums (PE broadcast), recip, normalize
                ex = exp_.tile([N, 4 * 260], f32r, tag="ex")
                exn = exnp.tile([N, 4 * 260], bf16, tag="exn")
                rec = recp.tile([N, 4 * 260], f32, tag="rec")
                for g in range(4):
                    pd = pMED.tile([N, 260], f32, tag="med")
                    for i4 in range(4):
                        h = _head_of(g, i4)
                        a, ph = h // 2, (h % 2) * 64
                        nc.tensor.matmul(
                            pd[:, i4 * 65:(i4 + 1) * 65],
                            kv[ph:ph + 64, a, toff:toff + N],
                            qv[ph:ph + 64, a, toff:toff + N],
                            start=True, stop=True,
                        )
                    gc = slice(g * 260, (g + 1) * 260)
                    nc.scalar.activation(ex[:, gc], pd[:], Act.Exp)
                # conv 3x3 SAME (f32r, no ldweights) + BN affine
                bn = bnp.tile([N, DIM], f32, tag="bn")
                pcs = [pBIG.tile([128, 512], f32, tag="big", name="pcs")
                       for _ in range(2)]
                pcys = [p[0:N, :].rearrange("c (h d) -> c h d", d=64) for p in pcs]
                for t in (4, 0, 1, 2, 3, 5, 6, 7, 8):
                    dy, dx = t // 3 - 1, t % 3 - 1
                    for half in range(2):
                        y0, y1 = half * 8, half * 8 + 8
                        oy0, oy1 = max(y0, -dy), min(y1, 16 - dy)
                        nc.tensor.matmul(
                            pcys[half][:, oy0 - y0:oy1 - y0, :],
                            R(wconv[:, t, :]),
                            R(vpfh[:, oy0 + dy:oy1 + dy, 1 + dx:65 + dx]),
                            start=(t == 4), stop=(t == 8),
                        )

                for g in range(4):
                    gc = slice(g * 260, (g + 1) * 260)
                    sm = pMED.tile([N, 260], f32, tag="med")
                    nc.tensor.matmul(sm[:], R(ones65[:]), R(ex[:, gc]),
                                     start=True, stop=True)
                    nc.vector.reciprocal(rec[:, gc], sm[:])
                    if g < 2:
                        nc.gpsimd.tensor_tensor(exn[:, gc], ex[:, gc], rec[:, gc],
                                                Alu.mult)
                    else:
                        nc.vector.tensor_tensor(exn[:, gc], ex[:, gc], rec[:, gc],
                                                Alu.mult)

                for half in range(2):
                    nc.vector.tensor_scalar(
                        bn[:, half * 512:half * 512 + 512], pcs[half][0:N, :],
                        st_sb[:, 0:1], st_sb[:, 1:2], Alu.mult, Alu.add,
                    )
                # fused: rt = (attn@v)^T + bn^T  per head pair
                for c8 in range(8):
                    bt = pAC.tile([128, 128], f32, tag="ac")
                    nc.tensor.matmul(
                        R(bt[:, 0:N]), R(bn[:, c8 * 128:(c8 + 1) * 128]),
                        R(idf_sb[:]),
                        is_transpose=True, start=True, stop=True,
                    )
                    btf = btp.tile([128, N], f32, tag="btf")
                    nc.scalar.copy(btf[:], bt[:, 0:N])
                    ac = pAC.tile([128, 128], f32, tag="ac")
                    he, ho = 2 * c8, 2 * c8 + 1
                    ce, co = _excol(he), _excol(ho)
                    nc.tensor.matmul(
                        ac[0:64, 0:N], vphh[:, he, 1:65], exn[:, ce:ce + 65],
                        start=True, stop=True,
                    )
                    nc.tensor.matmul(
                        ac[64:128, 0:N], vphh[:, ho, 1:65], exn[:, co:co + 65],
                        start=True, stop=True,
                    )
                    nc.vector.tensor_tensor(
                        rtv[:, c8, toff:toff + N], ac[:, 0:N], btf[:], Alu.add)

                # grouped final projection: 128-token tiles spanning batches
                while gdone * 128 + 128 <= (b + 1) * N:
                    emit_out_group(gdone * 128, 128)
                    gdone += 1
            if gdone * 128 < TOK:
                emit_out_group(gdone * 128, TOK - gdone * 128)
    return nc


def kernel(x, w_qkv, b_qkv, w_out, b_out, conv_w, conv_b,
           bn_gamma, bn_beta, bn_mean, bn_var):
    import os
    os.environ["BASS_NEVER_TRACE"] = "1"   # no NTFF hook in this container
    import concourse.bass as bass
    import concourse.bacc as bacc
    import concourse.mybir as mybir
    from concourse.bass_utils import run_bass_kernel_spmd
    import ml_dtypes

    bf = ml_dtypes.bfloat16
    scale = float(DIM) ** -0.5
    x = np.asarray(x, np.float32)
    wq = np.asarray(w_qkv, np.float32).copy()
    wq[:DIM] *= scale                                  # fold attn scale into W_q
    wqkvt = np.ascontiguousarray(wq.T)                 # [1024, 3072] f32
    import ml_dtypes
    woutt = np.ascontiguousarray(np.asarray(w_out, np.float32).T.astype(ml_dtypes.bfloat16))
    wconvt = np.ascontiguousarray(
        np.asarray(conv_w, np.float32).transpose(2, 3, 1, 0).reshape(9, N, N))
    s = np.asarray(bn_gamma, np.float32) / np.sqrt(
        np.asarray(bn_var, np.float32) + BN_EPS)
    t_aff = (np.asarray(conv_b, np.float32) - np.asarray(bn_mean, np.float32)) * s \
        + np.asarray(bn_beta, np.float32)
    st = np.ascontiguousarray(np.stack([s, t_aff], 1))

    b_qkv = np.asarray(b_qkv, np.float32)
    b_out = np.asarray(b_out, np.float32)
    has_bqkv = bool(np.any(b_qkv))
    has_bout = bool(np.any(b_out))

    nc = bacc.Bacc()
    _build(nc, mybir, bass, has_bqkv, has_bout)
    nc.finalize()

    xt_all = np.ascontiguousarray(x.reshape(B * N, DIM).T)

    base = {"wqkvt": wqkvt, "woutt": woutt, "wconvt": wconvt, "st": st,
            "ident": np.eye(128, dtype=bf),
            "identf": np.eye(N, dtype=np.float32)}
    if has_bqkv:
        bq = b_qkv.copy()
        bq[:DIM] *= scale
        base["bqkvc"] = np.ascontiguousarray(bq.reshape(24, 128).T)
    if has_bout:
        base["boutc"] = np.ascontiguousarray(b_out[None, :].astype(ml_dtypes.bfloat16))

    in_maps = []
    for c in range(NCORES):
        m = dict(base)
        m["xt"] = np.ascontiguousarray(xt_all[:, c * TOK:(c + 1) * TOK])
        in_maps.append(m)
    res = run_bass_kernel_spmd(nc, in_maps, list(range(NCORES)))
    globals()["LAST_RESULT"] = res
    outs = [res.results[c]["out"] for c in range(NCORES)]
    return np.concatenate(outs, axis=0).reshape(B, N, DIM).astype(np.float32)


# revision 72
# speedup vs baseline: 1.0896x; 1.0083x over previous
import numpy as np

# nn_Attention: B=256, N=65, DIM=1024, HEADS=16, DH=64 across 8 cores (32 batches/core)
B, N, DIM, HEADS, DH = 256, 65, 1024, 16, 64
NCORES = 8
BPC = B // NCORES            # 32 batches per core
TOK = BPC * N                # 2080 tokens per core
CH = 416                     # token-chunk for the qkv projection
NCHUNK = TOK // CH           # 5
BN_EPS = 1e-5


def _head_of(g, i4):
    # exp-group g holds 4 heads: g0={0,2,4,6} g1={1,3,5,7} g2={8,10,..} g3={9,11,..}
    if g < 2:
        return g + 2 * i4
    return 8 + (g - 2) + 2 * i4


def _excol(h):
    # column of head h inside the [N, 4*260] ex/exn/rec tiles
    if h < 8:
        return (h % 2) * 260 + (h // 2) * 65
    return 520 + (h % 2) * 260 + ((h - 8) // 2) * 65


def _build(nc_mod, mybir, bass, has_bqkv, has_bout):
    f32 = mybir.dt.float32
    f32r = mybir.dt.float32r
    bf16 = mybir.dt.bfloat16
    Alu = mybir.AluOpType
    Act = mybir.ActivationFunctionType
    from concourse.tile import TileContext
    from contextlib import ExitStack

    nc = nc_mod
    xt = nc.declare_dram_parameter("xt", [DIM, TOK], f32r, isOutput=False)
    wqkvt = nc.declare_dram_parameter("wqkvt", [DIM, 3 * DIM], f32r, isOutput=False)
    woutt = nc.declare_dram_parameter("woutt", [DIM, DIM], bf16, isOutput=False)
    wconvt = nc.declare_dram_parameter("wconvt", [9, N, N], f32r, isOutput=False)
    stp = nc.declare_dram_parameter("st", [N, 2], f32, isOutput=False)
    idp = nc.declare_dram_parameter("ident", [128, 128], bf16, isOutput=False)
    idfp = nc.declare_dram_parameter("identf", [N, N], f32, isOutput=False)
    if has_bqkv:
        bqp = nc.declare_dram_parameter("bqkvc", [128, 24], f32, isOutput=False)
    if has_bout:
        bop = nc.declare_dram_parameter("boutc", [1, DIM], bf16, isOutput=False)
    out = nc.declare_dram_parameter("out", [TOK, DIM], f32, isOutput=True)

    R = lambda ap: ap

    with TileContext(nc) as tc:
        with ExitStack() as es:
            P = lambda *a, **k: es.enter_context(tc.tile_pool(*a, **k))
            cp = P(name="consts", bufs=1)
            qkvp = P(name="qkv", bufs=1)

            id_sb = cp.tile([128, 128], bf16, tag="id")
            nc.sync.dma_start(out=id_sb[:], in_=idp[:])
            idf_sb = cp.tile([N, N], f32, tag="idf")
            nc.sync.dma_start(out=idf_sb[:], in_=idfp[:])
            wconv_sb = cp.tile([N, 9 * N], f32r, tag="wconv")
            nc.sync.dma_start(
                out=wconv_sb[:].rearrange("c (t o) -> c t o", t=9),
                in_=wconvt[:].rearrange("t c o -> c t o"),
            )
            wconv = wconv_sb[:].rearrange("c (t o) -> c t o", t=9)
            st_sb = cp.tile([N, 2], f32, tag="st")
            nc.sync.dma_start(out=st_sb[:], in_=stp[:])
            ones65 = cp.tile([N, N], f32r, tag="ones65")
            nc.gpsimd.memset(ones65[:].bitcast(f32), 1.0)
            if has_bqkv:
                bq_sb = cp.tile([128, 24], f32, tag="bq")
                nc.sync.dma_start(out=bq_sb[:], in_=bqp[:])
            if has_bout:
                bo_sb = cp.tile([1, DIM], bf16, tag="bo")
                nc.sync.dma_start(out=bo_sb[:], in_=bop[:])
                ones1 = cp.tile([1, 128], bf16, tag="ones1")
                nc.gpsimd.memset(ones1[:], 1.0)

            q_sb = qkvp.tile([128, 8 * TOK], bf16, tag="q")
            k_sb = qkvp.tile([128, 8 * TOK], bf16, tag="k")
            v_sb = qkvp.tile([128, 8 * TOK], bf16, tag="v")
            qv = q_sb[:].rearrange("p (a n) -> p a n", a=8)
            kv = k_sb[:].rearrange("p (a n) -> p a n", a=8)
            vv = v_sb[:].rearrange("p (a n) -> p a n", a=8)

            # ---- phase 1: qkv projection (feature-major, f32r), xt scoped ----
            with ExitStack() as es1:
                P1 = lambda *a, **k: es1.enter_context(tc.tile_pool(*a, **k))
                xtp = P1(name="xtp", bufs=1)
                wqp = P1(name="wqp", bufs=3)
                pP = P1(name="pP", bufs=5, space="PSUM")
                xt_sb = xtp.tile([128, 8 * TOK], f32r, tag="xt")
                xtv = xt_sb[:].rearrange("p (a n) -> p a n", a=8)
                xtd = xt[:].rearrange("(a p) n -> p a n", p=128)
                nc.sync.dma_start(
                    out=xtv[:, 0:2, 0:CH], in_=xtd[:, 0:2, 0:CH])
                wqd = wqkvt[:].rearrange("(ki p) f -> p ki f", p=128)

                for di, (dv, coff, boff) in enumerate(
                        ((vv, 2 * DIM, 16), (kv, DIM, 8), (qv, 0, 0))):
                    for a in range(8):
                        wqb = wqp.tile([128, 8 * 128], f32r, tag="wqb")
                        wqbv = wqb[:].rearrange("p (ki f) -> p ki f", ki=8)
                        nc.sync.dma_start(
                            out=wqbv,
                            in_=wqd[:, :, coff + a * 128:coff + (a + 1) * 128],
                        )
                        if di == 0 and a == 0:
                            # rest of x streams in behind the first weight block
                            nc.sync.dma_start(
                                out=xtv[:, 2:5, 0:CH], in_=xtd[:, 2:5, 0:CH])
                            nc.sync.dma_start(
                                out=xtv[:, 5:8, 0:CH], in_=xtd[:, 5:8, 0:CH])
                            for c in range(1, NCHUNK):
                                nc.sync.dma_start(
                                    out=xtv[:, :, c * CH:(c + 1) * CH],
                                    in_=xtd[:, :, c * CH:(c + 1) * CH],
                                )
                        for c in range(NCHUNK):
                            ps = pP.tile([128, CH], f32, tag="pp")
                            for ki in range(8):
                                nc.tensor.matmul(
                                    ps[:],
                                    R(wqbv[:, ki, :]),
                                    R(xtv[:, ki, c * CH:(c + 1) * CH]),
                                    start=(ki == 0), stop=(ki == 7),
                                )
                            o = dv[:, a, c * CH:(c + 1) * CH]
                            if has_bqkv:
                                nc.vector.tensor_scalar_add(
                                    o, ps[:], bq_sb[:, boff + a:boff + a + 1])
                            elif (a + c) % 2 == 0:
                                nc.vector.tensor_copy(o, ps[:])
                            else:
                                nc.scalar.copy(o, ps[:])

            # ---- phase 2: attention + conv + out projection per batch ----
            wop = P(name="wop", bufs=1)
            wo_sb = wop.tile([128, 8 * DIM], bf16, tag="wo")
            nc.sync.dma_start(
                out=wo_sb[:].rearrange("p (a n) -> p a n", a=8),
                in_=woutt[:].rearrange("(a p) n -> p a n", p=128),
            )
            wo = wo_sb[:].rearrange("p (a n) -> p a n", a=8)

            vphp = P(name="vphp", bufs=3)
            vpfp = P(name="vpfp", bufs=2)
            exp_ = P(name="exp", bufs=2)
            exnp = P(name="exnp", bufs=3)
            recp = P(name="recp", bufs=3)
            rtp = P(name="rtp", bufs=1)
            btp = P(name="btp", bufs=3)
            bnp = P(name="bnp", bufs=2)
            obp = P(name="obp", bufs=3)
            rta = rtp.tile([128, 8 * TOK], bf16, tag="rta")
            rtv = rta[:].rearrange("p (a n) -> p a n", a=8)
            pMED = P(name="pmed", bufs=2, space="PSUM")
            pPT = P(name="ppt", bufs=2, space="PSUM")
            pAC = P(name="pac", bufs=2, space="PSUM")
            pBIG = P(name="pbig", bufs=2, space="PSUM")

            def emit_out_group(r0, M):
                for half in range(2):
                    po = pBIG.tile([128, 512], f32, tag="big", name="po")
                    for ki in range(8):
                        nc.tensor.matmul(
                            po[0:M, :],
                            rtv[:, ki, r0:r0 + M],
                            wo[:, ki, half * 512:half * 512 + 512],
                            start=(ki == 0), stop=(ki == 7 and not has_bout),
                        )
                    if has_bout:
                        nc.tensor.matmul(
                            po[0:M, :], ones1[:, 0:M],
                            bo_sb[:, half * 512:half * 512 + 512],
                            start=False, stop=True,
                        )
                    ob = obp.tile([128, 512], f32, tag="ob")
                    if half == 0:
                        nc.vector.tensor_copy(ob[0:M, :], po[0:M, :])
                    else:
                        nc.scalar.copy(ob[0:M, :], po[0:M, :])
                    nc.sync.dma_start(
                        out=out[r0:r0 + M, half * 512:half * 512 + 512],
                        in_=ob[0:M, :],
                    )

            def stage1_vimg(b):
                # v for batch b, token-major padded image [c=65, 16 rows, 66]
                # bf16 copy feeds attn@v, f32 copy feeds the f32r conv
                toff = b * N
                vph = vphp.tile([N, HEADS * 66], bf16, tag="vph", name="vph")
                vphh = vph[:].rearrange("c (h w) -> c h w", w=66)
                vpf = vpfp.tile([N, HEADS * 66], f32r, tag="vpf", name="vpf")
                vpfh = vpf[:].rearrange("c (h w) -> c h w", w=66)
                nc.gpsimd.memset(vphh[:, :, 0:1], 0.0)
                nc.gpsimd.memset(vphh[:, :, 65:66], 0.0)
                nc.gpsimd.memset(vpfh[:, :, 0:1].bitcast(f32), 0.0)
                nc.gpsimd.memset(vpfh[:, :, 65:66].bitcast(f32), 0.0)
                for a in range(8):
                    pt = pPT.tile([128, 128], bf16, tag="pt", name="pt")
                    nc.tensor.transpose(pt[0:N, :], vv[:, a, toff:toff + N], id_sb[:])
                    src = pt[0:N, :].rearrange("c (h d) -> c h d", d=64)
                    if a % 2 == 0:
                        nc.vector.tensor_copy(vphh[:, 2 * a:2 * a + 2, 1:65], src)
                        nc.scalar.copy(vpfh[:, 2 * a:2 * a + 2, 1:65], src)
                    else:
                        nc.scalar.copy(vphh[:, 2 * a:2 * a + 2, 1:65], src)
                        nc.vector.tensor_copy(vpfh[:, 2 * a:2 * a + 2, 1:65], src)
                return vphh, vpfh

            gdone = 0
            vimg = stage1_vimg(0)
            for b in range(BPC):
                toff = b * N
                vphh, vpfh = vimg

                # attention scores, exp, rowsums (PE broadcast), recip, normalize
                ex = exp_.tile([N, 4 * 260], f32r, tag="ex")
                exn = exnp.tile([N, 4 * 260], bf16, tag="exn")
                rec = recp.tile([N, 4 * 260], f32, tag="rec")
                for g in range(4):
                    pd = pMED.tile([N, 260], f32, tag="med")
                    for i4 in range(4):
                        h = _head_of(g, i4)
                        a, ph = h // 2, (h % 2) * 64
                        nc.tensor.matmul(
                            pd[:, i4 * 65:(i4 + 1) * 65],
                            kv[ph:ph + 64, a, toff:toff + N],
                            qv[ph:ph + 64, a, toff:toff + N],
                            start=True, stop=True,
                        )
                    gc = slice(g * 260, (g + 1) * 260)
                    nc.scalar.activation(ex[:, gc], pd[:], Act.Exp)
                # conv 3x3 SAME (f32r, no ldweights) + BN affine
                bn = bnp.tile([N, DIM], f32, tag="bn")
                pcs = [pBIG.tile([128, 512], f32, tag="big", name="pcs")
                       for _ in range(2)]
                pcys = [p[0:N, :].rearrange("c (h d) -> c h d", d=64) for p in pcs]
                for t in (4, 0, 1, 2, 3, 5, 6, 7, 8):
                    dy, dx = t // 3 - 1, t % 3 - 1
                    for half in range(2):
                        y0, y1 = half * 8, half * 8 + 8
                        oy0, oy1 = max(y0, -dy), min(y1, 16 - dy)
                        nc.tensor.matmul(
                            pcys[half][:, oy0 - y0:oy1 - y0, :],
                            R(wconv[:, t, :]),
                            R(vpfh[:, oy0 + dy:oy1 + dy, 1 + dx:65 + dx]),
                            start=(t == 4), stop=(t == 8),
                        )

                for g in range(4):
                    gc = slice(g * 260, (g + 1) * 260)
                    sm = pMED.tile([N, 260], f32, tag="med")
                    nc.tensor.matmul(sm[:], R(ones65[:]), R(ex[:, gc]),
                                     start=True, stop=True)
                    nc.vector.reciprocal(rec[:, gc], sm[:])
                    if g < 2:
                        nc.gpsimd.tensor_tensor(exn[:, gc], ex[:, gc], rec[:, gc],
                                                Alu.mult)
                    else:
                        nc.vector.tensor_tensor(exn[:, gc], ex[:, gc], rec[:, gc],
                                                Alu.mult)

                if b + 1 < BPC:
                    vimg = stage1_vimg(b + 1)
                for half in range(2):
                    nc.vector.tensor_scalar(
                        bn[:, half * 512:half * 512 + 512], pcs[half][0:N, :],
                        st_sb[:, 0:1], st_sb[:, 1:2], Alu.mult, Alu.add,
                    )
                # fused: rt = (attn@v)^T + bn^T  per head pair
                for c8 in range(8):
                    bt = pAC.tile([128, 128], f32, tag="ac")
                    nc.tensor.matmul(
                        R(bt[:, 0:N]), R(bn[:, c8 * 128:(c8 + 1) * 128]),
                        R(idf_sb[:]),
                        is_transpose=True, start=True, stop=True,
                    )
                    btf = btp.tile([128, N], f32, tag="btf")
                    nc.scalar.copy(btf[:], bt[:, 0:N])
                    ac = pAC.tile([128, 128], f32, tag="ac")
                    he, ho = 2 * c8, 2 * c8 + 1
                    ce, co = _excol(he), _excol(ho)
                    nc.tensor.matmul(
                        ac[0:64, 0:N], vphh[:, he, 1:65], exn[:, ce:ce + 65],
                        start=True, stop=True,
                    )
                    nc.tensor.matmul(
                        ac[64:128, 0:N], vphh[:, ho, 1:65], exn[:, co:co + 65],
                        start=True, stop=True,
                    )
                    nc.vector.tensor_tensor(
                        rtv[:, c8, toff:toff + N], ac[:, 0:N], btf[:], Alu.add)

                # grouped final projection: 128-token tiles spanning batches
                while gdone * 128 + 128 <= (b + 1) * N:
                    emit_out_group(gdone * 128, 128)
                    gdone += 1
            if gdone * 128 < TOK:
                emit_out_group(gdone * 128, TOK - gdone * 128)
    return nc


def kernel(x, w_qkv, b_qkv, w_out, b_out, conv_w, conv_b,
           bn_gamma, bn_beta, bn_mean, bn_var):
    import os
    os.environ["BASS_NEVER_TRACE"] = "1"   # no NTFF hook in this container
    import concourse.bass as bass
    import concourse.bacc as bacc
    import concourse.mybir as mybir
    from concourse.bass_utils import run_bass_kernel_spmd
    import ml_dtypes

    bf = ml_dtypes.bfloat16
    scale = float(DIM) ** -0.5
    x = np.asarray(x, np.float32)
    wq = np.asarray(w_qkv, np.float32).copy()
    wq[:DIM] *= scale                                  # fold attn scale into W_q
    wqkvt = np.ascontiguousarray(wq.T)                 # [1024, 3072] f32
    import ml_dtypes
    woutt = np.ascontiguousarray(np.asarray(w_out, np.float32).T.astype(ml_dtypes.bfloat16))
    wconvt = np.ascontiguousarray(
        np.asarray(conv_w, np.float32).transpose(2, 3, 1, 0).reshape(9, N, N))
    s = np.asarray(bn_gamma, np.float32) / np.sqrt(
        np.asarray(bn_var, np.float32) + BN_EPS)
    t_aff = (np.asarray(conv_b, np.float32) - np.asarray(bn_mean, np.float32)) * s \
        + np.asarray(bn_beta, np.float32)
    st = np.ascontiguousarray(np.stack([s, t_aff], 1))

    b_qkv = np.asarray(b_qkv, np.float32)
    b_out = np.asarray(b_out, np.float32)
    has_bqkv = bool(np.any(b_qkv))
    has_bout = bool(np.any(b_out))

    nc = bacc.Bacc()
    _build(nc, mybir, bass, has_bqkv, has_bout)
    nc.finalize()

    xt_all = np.ascontiguousarray(x.reshape(B * N, DIM).T)

    base = {"wqkvt": wqkvt, "woutt": woutt, "wconvt": wconvt, "st": st,
            "ident": np.eye(128, dtype=bf),
            "identf": np.eye(N, dtype=np.float32)}
    if has_bqkv:
        bq = b_qkv.copy()
        bq[:DIM] *= scale
        base["bqkvc"] = np.ascontiguousarray(bq.reshape(24, 128).T)
    if has_bout:
        base["boutc"] = np.ascontiguousarray(b_out[None, :].astype(ml_dtypes.bfloat16))

    in_maps = []
    for c in range(NCORES):
        m = dict(base)
        m["xt"] = np.ascontiguousarray(xt_all[:, c * TOK:(c + 1) * TOK])
        in_maps.append(m)
    res = run_bass_kernel_spmd(nc, in_maps, list(range(NCORES)))
    globals()["LAST_RESULT"] = res
    outs = [res.results[c]["out"] for c in range(NCORES)]
    return np.concatenate(outs, axis=0).reshape(B, N, DIM).astype(np.float32)
